# revision 6
# baseline (speedup 1.0000x reference)
"""Causal self-attention Trainium2 kernel (8 NeuronCores, SPMD).

Problem: B=2, T=2048, D=1024, H=16 heads (head_dim 64), fp32 I/O.
    qkv = x @ Wqkv + bqkv ; per-head causal softmax(q k^T / 8) @ v ; out @ Wout + bout

Sharding: 2 batch groups x 4 cores. Core c: batch b=c//4, head group g=c%4
(heads 4g..4g+3, i.e. D-slice [256g, 256g+256)), and out-proj column slice
[256g, 256g+256). Attention outputs are AllGathered (bf16) within each
4-core batch group per query chunk; out-proj is column-sharded so the
final output needs no reduction -- each core returns a [256, 2048] slice
(transposed) which the host reassembles.

Engine budget (per core): ACT exp ~8.9M elems ~= 90us is the steady-state
bottleneck; PE ~220K matmul-slot columns ~= 95-115us; the serial CC stream
(entry barrier ~50us one-time + ~15-25us per AllGather) is the tail
bottleneck.  The schedule therefore optimizes (a) time-to-first-exp,
(b) exp-stream density, (c) hiding the last AllGather behind deferred
out-proj work.

Key scheduling decisions (vs a naive emission):
  - DMA order puts wq + x-chunk0 + wk first so qk(0) can start ~10us in;
    a dummy exp preloads the ACT table (1.3us) during the DMA window;
    PE warmup is ~14 N=256 matmuls sized to the DMA window (HAM un-throttle
    needs ~3.4us of activity, and oversizing delays qk0 in the PE FIFO).
  - qkv groups for chunk c+1 are emitted as 4-matmul HALF-groups paced
    into attention(c)'s per-key-tile slots by a column budget, so the
    S(kk+1) matmuls (which gate the next exp) never sit behind >1us of
    filler in the PE FIFO.
  - PV psum is copied to SBUF (DVE) right after the last PV matmul and
    the softmax normalize runs from SBUF, so the 2 PV psum banks free in
    ~1us instead of ~5us and the next chunk's PV never head-of-line
    blocks the S stream at a chunk boundary.
  - ALL out-proj groups are deferred until after the LAST AllGather
    trigger: proj(0..3) (~13us of PE work, data-ready) then fill the
    ~16us latency of AG(4); only proj(4) (0.9us) is exposed.
  - The last 512 tokens are split 384+128 and the 128-wide chunk uses the
    "narrow" path: all 4 heads' S in one psum tile -> ONE exp per key
    tile (halves ACT instruction overhead where ACT overhead dominates).
  - A dummy 1KB AllGather is triggered first thing: the collectives stack
    pays its ~50us one-time init/entry-barrier during the compute head
    instead of on the first real AllGather.
"""

import numpy as np
import ml_dtypes

import concourse.bass as bass
import concourse.tile as tile
from concourse import bacc, bass_utils, mybir

BF16 = mybir.dt.bfloat16
F32 = mybir.dt.float32

B, T, D, H = 2, 2048, 1024, 16
HD = D // H  # 64
NCORES = 8
GROUPS = [[0, 1, 2, 3], [4, 5, 6, 7]]
P = 128  # partitions
FS = D // P  # 8 feature slices
NTC = T // 512  # 4 key/token chunks (k/v tiling; fixed)
DL = 256  # local d (4 heads * 64)
NMT = DL // P  # 2 stationary M-tiles

# query chunks: (q_lo, q_len); last 512 split 384+128 to shrink the tail
QCHUNKS = [(0, 512), (512, 512), (1024, 512), (1536, 384), (1920, 128)]
NQC = len(QCHUNKS)


def build_bass():
    nc = bacc.Bacc("TRN2", target_bir_lowering=False, debug=False,
                   num_devices=NCORES)

    xt_d = nc.dram_tensor("xt", [D, T], BF16, kind="ExternalInput")
    wq_d = nc.dram_tensor("wq", [D, DL], BF16, kind="ExternalInput")
    wk_d = nc.dram_tensor("wk", [D, DL], BF16, kind="ExternalInput")
    wv_d = nc.dram_tensor("wv", [D, DL], BF16, kind="ExternalInput")
    wo_d = nc.dram_tensor("wout", [D, DL], BF16, kind="ExternalInput")
    bq_d = nc.dram_tensor("bq", [P, NMT], F32, kind="ExternalInput")
    bk_d = nc.dram_tensor("bk", [P, NMT], F32, kind="ExternalInput")
    bv_d = nc.dram_tensor("bv", [P, DL], F32, kind="ExternalInput")
    bo_d = nc.dram_tensor("bo", [P, NMT], F32, kind="ExternalInput")
    tri_d = nc.dram_tensor("tri", [P, P], BF16, kind="ExternalInput")
    ones_d = nc.dram_tensor("ones", [P, 32], BF16, kind="ExternalInput")
    outT_d = nc.dram_tensor("outT", [DL, T], F32, kind="ExternalOutput")

    # chunks 0-1 share ONE AllGather (issued after chunk 1's normalize):
    # fewer collectives on the serial CC stream.
    ag_in = [nc.dram_tensor("ag_in01", [DL, 1024], BF16)] * 2 + [
        nc.dram_tensor(f"ag_in{ci}", [DL, ql], BF16)
        for ci, (_, ql) in list(enumerate(QCHUNKS))[2:]]
    ag_out = [nc.dram_tensor("ag_out01", [D, 1024], BF16)] * 2 + [
        nc.dram_tensor(f"ag_out{ci}", [D, ql], BF16)
        for ci, (_, ql) in list(enumerate(QCHUNKS))[2:]]
    ag_warm_in = nc.dram_tensor("ag_warm_in", [32, 16], BF16)
    ag_warm_out = nc.dram_tensor("ag_warm_out", [128, 16], BF16)

    with tile.TileContext(nc) as tc:
        with (
            tc.tile_pool(name="const", bufs=1) as const,
            tc.tile_pool(name="expst", bufs=4) as expst_pool,
            tc.tile_pool(name="attn", bufs=2) as attn_pool,
            tc.tile_pool(name="agf", bufs=3) as agf_pool,
            tc.tile_pool(name="outsb", bufs=2) as out_pool,
            tc.tile_pool(name="recip", bufs=2) as recip_pool,
            tc.tile_pool(name="ps_s", bufs=2, space="PSUM") as ps_s_pool,
            tc.tile_pool(name="ps_pv", bufs=2, space="PSUM") as ps_pv_pool,
            tc.tile_pool(name="ps_sum", bufs=1, space="PSUM") as ps_sum_pool,
            tc.tile_pool(name="ps_mm", bufs=1, space="PSUM") as ps_mm_pool,
        ):
            # ---- constant loads, ordered by first use -------------------
            xt_view = xt_d[:].rearrange("(s p) t -> p s t", p=P)
            wq_sb = const.tile([P, FS, DL], BF16)
            nc.sync.dma_start(wq_sb[:], wq_d[:].rearrange("(s p) n -> p s n", p=P))
            xt_tc = [const.tile([P, FS, 512], BF16, tag=f"xt{i}", name=f"xt{i}")
                     for i in range(NTC)]
            for s in range(FS):
                nc.sync.dma_start(xt_tc[0][:, s, :], xt_view[:, s, 0:512])
            wk_sb = const.tile([P, FS, DL], BF16)
            nc.sync.dma_start(wk_sb[:], wk_d[:].rearrange("(s p) n -> p s n", p=P))
            bq_sb = const.tile([P, NMT], F32)
            nc.sync.dma_start(bq_sb[:], bq_d[:])
            bk_sb = const.tile([P, NMT], F32)
            nc.sync.dma_start(bk_sb[:], bk_d[:])
            wv_sb = const.tile([P, FS, DL], BF16)
            nc.sync.dma_start(wv_sb[:], wv_d[:].rearrange("(s p) n -> p s n", p=P))
            bv_sb = const.tile([P, DL], F32)
            nc.sync.dma_start(bv_sb[:], bv_d[:])
            tri_sb = const.tile([P, P], BF16)
            nc.sync.dma_start(tri_sb[:], tri_d[:])
            ones_sb = const.tile([P, 32], BF16)
            nc.sync.dma_start(ones_sb[:], ones_d[:])
            zb = const.tile([P, 1], F32)
            nc.gpsimd.memset(zb[:], 0.0)
            # preload the ACT exp table (1.3us) during the DMA window
            dummy_sb = const.tile([P, 1], F32)
            nc.scalar.activation(dummy_sb[:], zb[:],
                                 mybir.ActivationFunctionType.Exp,
                                 bias=zb[:], scale=0.125)
            for tcidx in range(1, NTC):
                nc.sync.dma_start(xt_tc[tcidx][:],
                                  xt_view[:, :, 512 * tcidx:512 * tcidx + 512])
            wo_sb = const.tile([P, FS, DL], BF16)
            nc.sync.dma_start(wo_sb[:], wo_d[:].rearrange("(s p) n -> p s n", p=P))
            bo_sb = const.tile([P, NMT], F32)
            nc.sync.dma_start(bo_sb[:], bo_d[:])

            qT_tc = [const.tile([P, NMT, 512], BF16, tag=f"qT{i}", name=f"qT{i}") for i in range(NTC)]
            kT_tc = [const.tile([P, NMT, 512], BF16, tag=f"kT{i}", name=f"kT{i}") for i in range(NTC)]
            v_tc = [const.tile([P, 4, DL], BF16, tag=f"v{i}", name=f"v{i}") for i in range(NTC)]

            # ---- qkv emission: full groups (head) and half groups (fillers)
            def qk_emit(tcx, dst, w_sb, b_sb, mt, pool=None):
                """Returns two closures, each 4 matmuls; epilogue on the
                2nd. cols per half: 2048 (0.85us at 2.4GHz)."""
                xt = xt_tc[tcx]
                mmtile = (lambda: ps_mm_pool.tile([P, 512], F32, tag="mm",
                                                  name="mmps")) \
                    if pool is None else pool
                state = {}

                def half(h):
                    def emit(after=None):
                        if h == 0:
                            state["ps"] = mmtile()
                        ps = state["ps"]
                        for s in range(4 * h, 4 * h + 4):
                            mm = nc.tensor.matmul(
                                ps[:], w_sb[:, s, P * mt:P * mt + P],
                                xt[:, s, :],
                                start=(s == 0), stop=(s == FS - 1))
                            if after is not None and s == 4 * h:
                                tile.add_dep_helper(mm.ins, after, sync=False,
                                                    reason="filler order")
                        if h == 1:
                            nc.vector.tensor_scalar_add(
                                dst[:, mt, :], ps[:], b_sb[:, mt:mt + 1])
                    return emit
                return [half(0), half(1)]

            def v_emit(tcx, tt, vpool=None):
                """Two closures of 4 matmuls each (1024 cols per half).
                Consecutive v groups use alternating halves of the psum
                bank so tt+1's matmuls don't wait tt's epilogue."""
                xt = xt_tc[tcx]
                vtile = (lambda: ps_mm_pool.tile([P, 512], F32, tag="mm",
                                                 name="mmps")) \
                    if vpool is None else vpool
                lo = DL * (tt % 2)
                state = {}

                def half(h):
                    def emit(after=None):
                        if h == 0:
                            state["ps"] = vtile()
                        ps = state["ps"]
                        for s in range(4 * h, 4 * h + 4):
                            mm = nc.tensor.matmul(
                                ps[:, lo:lo + DL], xt[:, s, P * tt:P * tt + P],
                                wv_sb[:, s, :],
                                start=(s == 0), stop=(s == FS - 1))
                            if after is not None and s == 4 * h:
                                tile.add_dep_helper(mm.ins, after, sync=False,
                                                    reason="filler order")
                        if h == 1:
                            nc.vector.tensor_add(
                                v_tc[tcx][:, tt, :], ps[:, lo:lo + DL], bv_sb[:])
                    return emit
                return [half(0), half(1)]

            # filler queue entries: (cols, closure)
            Q_COLS, V_COLS = 2048, 1024

            def q_halves(tcx):
                out = []
                for mt in range(NMT):
                    for f in qk_emit(tcx, qT_tc[tcx], wq_sb, bq_sb, mt):
                        out.append((Q_COLS, f))
                return out

            def k_halves(tcx):
                out = []
                for mt in range(NMT):
                    for f in qk_emit(tcx, kT_tc[tcx], wk_sb, bk_sb, mt):
                        out.append((Q_COLS, f))
                return out

            def v_halves(tcx):
                out = []
                for tt in range(4):
                    for f in v_emit(tcx, tt):
                        out.append((V_COLS, f))
                return out

            def attention_chunk(ci, fillers=(), pin_fillers=False):
                """Attention for query chunk ci; returns the last PV matmul
                instruction (for pinning tail work behind it)."""
                q_lo, qlen = QCHUNKS[ci]
                q_tck, q_off0 = q_lo // 512, q_lo % 512
                nkk = (q_lo + qlen) // P
                fillers = list(fillers)
                ps_pv = [ps_pv_pool.tile([P, 512], F32, tag="pv",
                                         name=f"pv{ci}_{i}") for i in range(2)]
                ps_sum = ps_sum_pool.tile([P, 512], F32)
                last_mm = [None]

                def geom(kk):
                    tck, m = kk // 4, kk % 4
                    d = P * kk - q_lo
                    if d < 0:
                        return tck, m, False, 0, qlen
                    return tck, m, True, d, qlen - d

                # narrow chunks (q_len <= 128): all 4 heads' S fit in ONE
                # psum tile half-pair -> one exp instruction per key tile
                # instead of two (saves the 352-cycle ACT issue overhead).
                narrow = qlen <= 128

                def eslice(expst, h, n):
                    if narrow:
                        return expst[:, h % 2, qlen * (h // 2):
                                     qlen * (h // 2) + n]
                    return expst[:, h, 0:n]

                def emit_s_exp(kk):
                    """S^T in two 2-head halves, each its own 2-bank psum
                    tile (pool bufs=2) so S(g+1) issues while ACT still
                    reads exp(g)'s input."""
                    tck, m, diag, off, W = geom(kk)
                    expst = expst_pool.tile([P, 4, 512], BF16, tag="expst",
                                            name=f"expst{ci}_{kk}")
                    if narrow:
                        ps_s = ps_s_pool.tile([P, 2, 512], F32, tag="s",
                                              name=f"s{ci}_{kk}")
                        for h in range(4):
                            mt, rp = h // 2, 64 * (h % 2)
                            mm = nc.tensor.matmul(
                                ps_s[:, h % 2, qlen * (h // 2):
                                     qlen * (h // 2) + W],
                                kT_tc[tck][rp:rp + 64, mt, P * m:P * m + P],
                                qT_tc[q_tck][rp:rp + 64, mt,
                                             q_off0 + off:q_off0 + off + W],
                                start=True, stop=True)
                            last_mm[0] = mm.ins
                        nc.scalar.activation(
                            expst[:, 0:2, 0:2 * qlen],
                            ps_s[:, 0:2, 0:2 * qlen],
                            mybir.ActivationFunctionType.Exp,
                            bias=zb[:], scale=0.125)
                    else:
                        for hp in range(2):
                            ps_s = ps_s_pool.tile([P, 2, 512], F32, tag="s",
                                                  name=f"s{ci}_{kk}_{hp}")
                            for hh in range(2):
                                h = 2 * hp + hh
                                mt, rp = h // 2, 64 * (h % 2)
                                mm = nc.tensor.matmul(
                                    ps_s[:, hh, 0:W],
                                    kT_tc[tck][rp:rp + 64, mt,
                                               P * m:P * m + P],
                                    qT_tc[q_tck][rp:rp + 64, mt,
                                                 q_off0 + off:q_off0 + off + W],
                                    start=True, stop=True)
                                last_mm[0] = mm.ins
                            nc.scalar.activation(
                                expst[:, 2 * hp:2 * hp + 2, 0:W],
                                ps_s[:, 0:2, 0:W],
                                mybir.ActivationFunctionType.Exp,
                                bias=zb[:], scale=0.125)
                    if diag:
                        # SBUF-only bf16 muls -> idle GpSimd, keeping DVE
                        # free for the psum-slot-releasing epilogues
                        for h in range(4):
                            nc.gpsimd.tensor_mul(
                                eslice(expst, h, P),
                                eslice(expst, h, P), tri_sb[:])
                    return expst

                def emit_pv_sums(kk, expst):
                    tck, m, diag, off, W = geom(kk)
                    # PV^T accumulation (V stationary, exp moving), 2 heads/slot
                    for hp in range(2):
                        for hh in range(2):
                            h = 2 * hp + hh
                            mm = nc.tensor.matmul(
                                ps_pv[hp][64 * hh:64 * hh + 64, off:off + W],
                                v_tc[tck][:, m, 64 * h:64 * h + 64],
                                eslice(expst, h, W),
                                start=(kk == 0), stop=(kk == nkk - 1))
                            last_mm[0] = mm.ins
                    # softmax denominators: ones-matmuls (M=32 so each head's
                    # sum lands replicated on 32 partitions), 4 heads packed
                    # by 32-aligned column groups (partitions 32h..32h+31)
                    for h in range(4):
                        nc.tensor.matmul(
                            ps_sum[32 * h:32 * h + 32, off:off + W],
                            ones_sb[:, 0:32],
                            eslice(expst, h, W),
                            start=(kk == 0), stop=(kk == nkk - 1),
                            tile_position=(0, 32 * h))

                def pop_fillers():
                    # column budget per key-tile slot so the next S pair
                    # (which gates the next exp) is never far back in the
                    # PE FIFO.  Looser budget when the queue is backlogged.
                    budget = 2048 if len(fillers) <= (nkk - 1) else 3072
                    used = 0
                    while fillers and used + fillers[0][0] <= budget:
                        cols, f = fillers.pop(0)
                        used += cols
                        f(last_mm[0] if pin_fillers else None)

                # Software-pipelined emission: S+exp for kk+1 go into the
                # engine queues BEFORE PV/sums for kk; fillers sit between
                # S(kk+1) and PV(kk) so they can only delay PV accumulation
                # (slack) and never the S->exp chain.
                expst_prev = emit_s_exp(0)
                for kk in range(1, nkk):
                    expst_cur = emit_s_exp(kk)
                    pop_fillers()
                    emit_pv_sums(kk - 1, expst_prev)
                    expst_prev = expst_cur
                emit_pv_sums(nkk - 1, expst_prev)
                last_pv = last_mm[0]
                # flush remaining fillers (PE work overlapping the last exp
                # + the normalize chain below)
                while fillers:
                    _, f = fillers.pop(0)
                    f(last_mm[0] if pin_fillers else None)
                # normalize + stage for the AllGather. Chain the muls with
                # no-sync deps so hp0 finishes (and releases its PV psum
                # slot for the next chunk) before hp1 starts.
                recip = recip_pool.tile([P, 512], F32)
                nc.vector.reciprocal_approx_fast(recip[:, 0:qlen],
                                                 ps_sum[:, 0:qlen])
                prev_mul = None
                for hp in range(2):
                    attn = attn_pool.tile([P, 512], BF16)
                    for hh in range(2):
                        h = 2 * hp + hh
                        for half in range(2):
                            lo = 64 * hh + 32 * half
                            mul = nc.vector.tensor_mul(
                                attn[lo:lo + 32, 0:qlen],
                                ps_pv[hp][lo:lo + 32, 0:qlen],
                                recip[32 * h:32 * h + 32, 0:qlen])
                            if prev_mul is not None:
                                tile.add_dep_helper(
                                    mul.ins, prev_mul.ins, sync=False,
                                    reason="normalize order hp0-first")
                            prev_mul = mul
                    nc.sync.dma_start(
                        ag_in[ci][P * hp:P * hp + P,
                                  q_lo - QCHUNKS[AG_HEAD[ci]][0]:
                                  q_lo - QCHUNKS[AG_HEAD[ci]][0] + qlen],
                        attn[:, 0:qlen])
                if ci == AG_TAIL[ci]:
                    ag_chunk(ci)
                return last_pv

            # AG groups: chunks 0-1 share one collective (head chunk 0,
            # triggered after tail chunk 1); 2, 3, 4 are their own.
            AG_HEAD = [0, 0, 2, 3, 4]
            AG_TAIL = [1, 1, 2, 3, 4]
            agf_tiles = {}

            def ag_chunk(ci):
                glen = sum(QCHUNKS[c][1] for c in range(NQC)
                           if AG_HEAD[c] == AG_HEAD[ci])
                nc.gpsimd.collective_compute(
                    "AllGather", mybir.AluOpType.bypass,
                    replica_groups=GROUPS,
                    ins=[ag_in[ci][:]], outs=[ag_out[ci][:]])
                if ci <= 1:  # one-shot big tile; const pool (bufs=1)
                    agf = const.tile([P, FS, 1024], BF16, tag="agf01",
                                     name="agf01")
                else:
                    agf = agf_pool.tile([P, FS, 512], BF16, name=f"agf{ci}")
                # two half DMAs: proj can start on the first half while the
                # second lands, without paying 8 separate DMA-issue costs
                agv = ag_out[ci][:].rearrange("(s p) t -> p s t", p=P)
                nc.sync.dma_start(agf[:, 0:4, 0:glen], agv[:, 0:4, :])
                nc.sync.dma_start(agf[:, 4:8, 0:glen], agv[:, 4:8, :])
                agf_tiles[AG_HEAD[ci]] = agf

            def proj_groups(ci):
                q_lo, qlen = QCHUNKS[ci]
                agoff = q_lo - QCHUNKS[AG_HEAD[ci]][0]

                def group(mt):
                    def emit(after=None):
                        agf = agf_tiles[AG_HEAD[ci]]
                        ps = ps_mm_pool.tile([P, 512], F32, tag="mm")
                        for s in range(FS):
                            mm = nc.tensor.matmul(
                                ps[:, 0:qlen],
                                wo_sb[:, s, P * mt:P * mt + P],
                                agf[:, s, agoff:agoff + qlen],
                                start=(s == 0), stop=(s == FS - 1))
                            if after is not None and s == 0:
                                tile.add_dep_helper(mm.ins, after, sync=False,
                                                    reason="proj after attn")
                        osb = out_pool.tile([P, 512], F32)
                        if mt == 1:  # alternate epilogue engine (ACT idle in tail)
                            nc.scalar.add(osb[:, 0:qlen], ps[:, 0:qlen],
                                          bo_sb[:, mt:mt + 1])
                        else:
                            nc.vector.tensor_scalar_add(
                                osb[:, 0:qlen], ps[:, 0:qlen],
                                bo_sb[:, mt:mt + 1])
                        nc.sync.dma_start(
                            outT_d[P * mt:P * mt + P, q_lo:q_lo + qlen],
                            osb[:, 0:qlen])
                    return emit
                return [group(mt) for mt in range(NMT)]

            # ---- emission schedule --------------------------------------
            # PE warmup sized to the input-DMA window (~3us): HAM needs
            # ~3.4us of activity to un-throttle; oversizing delays qk0.
            warm_sb = const.tile([P, 256], BF16)
            nc.gpsimd.memset(warm_sb[:], 0.0)
            ps_w = ps_mm_pool.tile([P, 512], F32, tag="mm")
            for _ in range(14):
                nc.tensor.matmul(ps_w[:, 0:256], warm_sb[:, 0:P], warm_sb[:],
                                 start=True, stop=True)
            # dummy collective: the collectives stack pays its one-time
            # init + entry barrier (~50us) during the compute head
            nc.gpsimd.collective_compute(
                "AllGather", mybir.AluOpType.bypass, replica_groups=GROUPS,
                ins=[ag_warm_in[:]], outs=[ag_warm_out[:]])

            # chunk-0 q/k double-buffer through the ps_s slots and v(0)
            # through the (still idle) ps_pv slots -- two independent psum
            # chains run concurrently before attention(0) starts.
            spool = lambda: ps_s_pool.tile(
                [P, 2, 512], F32, tag="s", name="qkv0mm")[:, 0, :]
            vpool = lambda: ps_pv_pool.tile([P, 512], F32, tag="pv",
                                            name="qkv0v")
            for mt in range(NMT):
                for f in qk_emit(0, qT_tc[0], wq_sb, bq_sb, mt, pool=spool):
                    f()
            for mt in range(NMT):
                for f in qk_emit(0, kT_tc[0], wk_sb, bk_sb, mt, pool=spool):
                    f()
            for f in v_emit(0, 0, vpool=vpool):
                f()

            # per-chunk filler queues (half-group granularity), paced so
            # producers finish a few key-tiles before their consumers:
            #   c0: v0 tt1-3 then q1      (PV(c0,kk) needs v0 tt=kk)
            #   c1: k1, v1, q2            (S(c1,kk>=4) needs k1; PV needs v1)
            #   c2: k2, v2, q3
            #   c3: k3, v3
            #   c4: none (pure attention tail)
            f_c0 = [(V_COLS, f) for tt in (1, 2, 3) for f in v_emit(0, tt)] \
                + q_halves(1)
            attention_chunk(0, f_c0)
            f_c1 = k_halves(1) + v_halves(1) + q_halves(2)
            attention_chunk(1, f_c1)
            f_c2 = k_halves(2) + v_halves(2) + q_halves(3)
            attention_chunk(2, f_c2)
            f_c3 = k_halves(3) + v_halves(3)
            attention_chunk(3, f_c3)
            last_pv = attention_chunk(4)
            # ALL out-proj deferred here: proj(0..3) are data-ready and
            # fill the last AllGather's latency; only proj(4) waits on it.
            for ci in range(NQC):
                for g in proj_groups(ci):
                    g(last_pv)

    nc.compile()
    return nc


_NC_CACHE = None


def _get_nc():
    global _NC_CACHE
    if _NC_CACHE is None:
        _NC_CACHE = build_bass()
    return _NC_CACHE


def _make_in_maps(x, Wqkv, bqkv, Wout, bout):
    bf16 = ml_dtypes.bfloat16
    in_maps = []
    for c in range(NCORES):
        b, g = c // 4, c % 4
        cs = DL * g  # column/dim slice start for this core's heads
        im = {
            "xt": np.ascontiguousarray(x[b].T).astype(bf16),
            "wq": np.ascontiguousarray(Wqkv[:, cs:cs + DL]).astype(bf16),
            "wk": np.ascontiguousarray(Wqkv[:, D + cs:D + cs + DL]).astype(bf16),
            "wv": np.ascontiguousarray(Wqkv[:, 2 * D + cs:2 * D + cs + DL]).astype(bf16),
            "wout": np.ascontiguousarray(Wout[:, cs:cs + DL]).astype(bf16),
            "bq": np.ascontiguousarray(
                bqkv[cs:cs + DL].reshape(NMT, P).T).astype(np.float32),
            "bk": np.ascontiguousarray(
                bqkv[D + cs:D + cs + DL].reshape(NMT, P).T).astype(np.float32),
            "bv": np.ascontiguousarray(np.broadcast_to(
                bqkv[2 * D + cs:2 * D + cs + DL].reshape(1, DL),
                (P, DL))).astype(np.float32),
            "bo": np.ascontiguousarray(
                bout[cs:cs + DL].reshape(NMT, P).T).astype(np.float32),
            "tri": np.triu(np.ones((P, P))).astype(bf16),
            "ones": np.ones((P, 32), dtype=bf16),
        }
        in_maps.append(im)
    return in_maps


def _run(inputs, trace=False, tmpdir=None):
    nc = _get_nc()
    in_maps = _make_in_maps(**inputs)
    res = bass_utils.run_bass_kernel_spmd(
        nc, in_maps, core_ids=list(range(NCORES)), trace=trace, tmpdir=tmpdir)
    out = np.empty((B, T, D), dtype=np.float32)
    for c in range(NCORES):
        b, g = c // 4, c % 4
        out[b, :, DL * g:DL * g + DL] = res.results[c]["outT"].T
    return out, res


def kernel(x, Wqkv, bqkv, Wout, bout):
    out, _ = _run(dict(x=np.asarray(x, dtype=np.float32),
                       Wqkv=np.asarray(Wqkv, dtype=np.float32),
                       bqkv=np.asarray(bqkv, dtype=np.float32),
                       Wout=np.asarray(Wout, dtype=np.float32),
                       bout=np.asarray(bout, dtype=np.float32)))
    return out


# revision 16
# speedup vs baseline: 1.0402x; 1.0402x over previous
"""Causal self-attention Trainium2 kernel (8 NeuronCores, SPMD).

Problem: B=2, T=2048, D=1024, H=16 heads (head_dim 64), fp32 I/O.
    qkv = x @ Wqkv + bqkv ; per-head causal softmax(q k^T / 8) @ v ; out @ Wout + bout

Sharding: 2 batch groups x 4 cores. Core c: batch b=c//4, head group g=c%4
(heads 4g..4g+3, i.e. D-slice [256g, 256g+256)), and out-proj column slice
[256g, 256g+256). Attention outputs are AllGathered (bf16) within each
4-core batch group per query chunk; out-proj is column-sharded so the
final output needs no reduction -- each core returns a [256, 2048] slice
(transposed) which the host reassembles.

Engine budget (per core): ACT exp ~8.9M elems ~= 90us is the steady-state
bottleneck; PE ~220K matmul-slot columns ~= 95-115us; the serial CC stream
(entry barrier ~50us one-time + ~15-25us per AllGather) is the tail
bottleneck.  The schedule therefore optimizes (a) time-to-first-exp,
(b) exp-stream density, (c) hiding the last AllGather behind deferred
out-proj work.

Key scheduling decisions (vs a naive emission):
  - DMA order puts wq + x-chunk0 + wk first so qk(0) can start ~10us in;
    a dummy exp preloads the ACT table (1.3us) during the DMA window;
    PE warmup is ~14 N=256 matmuls sized to the DMA window (HAM un-throttle
    needs ~3.4us of activity, and oversizing delays qk0 in the PE FIFO).
  - qkv groups for chunk c+1 are emitted as 4-matmul HALF-groups paced
    into attention(c)'s per-key-tile slots by a column budget, so the
    S(kk+1) matmuls (which gate the next exp) never sit behind >1us of
    filler in the PE FIFO.
  - PV psum is copied to SBUF (DVE) right after the last PV matmul and
    the softmax normalize runs from SBUF, so the 2 PV psum banks free in
    ~1us instead of ~5us and the next chunk's PV never head-of-line
    blocks the S stream at a chunk boundary.
  - ALL out-proj groups are deferred until after the LAST AllGather
    trigger: proj(0..3) (~13us of PE work, data-ready) then fill the
    ~16us latency of AG(4); only proj(4) (0.9us) is exposed.
  - The last 512 tokens are split 384+128 and the 128-wide chunk uses the
    "narrow" path: all 4 heads' S in one psum tile -> ONE exp per key
    tile (halves ACT instruction overhead where ACT overhead dominates).
  - A dummy 1KB AllGather is triggered first thing: the collectives stack
    pays its ~50us one-time init/entry-barrier during the compute head
    instead of on the first real AllGather.
"""

import numpy as np
import ml_dtypes

import concourse.bass as bass
import concourse.tile as tile
from concourse import bacc, bass_utils, mybir

BF16 = mybir.dt.bfloat16
F32 = mybir.dt.float32

B, T, D, H = 2, 2048, 1024, 16
HD = D // H  # 64
NCORES = 8
GROUPS = [[0, 1, 2, 3], [4, 5, 6, 7]]
P = 128  # partitions
FS = D // P  # 8 feature slices
NTC = T // 512  # 4 key/token chunks (k/v tiling; fixed)
DL = 256  # local d (4 heads * 64)
NMT = DL // P  # 2 stationary M-tiles

# query chunks: (q_lo, q_len); last 512 split 384+128 to shrink the tail
QCHUNKS = [(0, 512), (512, 512), (1024, 512), (1536, 384), (1920, 128)]
NQC = len(QCHUNKS)


def build_bass():
    nc = bacc.Bacc("TRN2", target_bir_lowering=False, debug=False,
                   num_devices=NCORES)

    xt_d = nc.dram_tensor("xt", [D, T], BF16, kind="ExternalInput")
    wq_d = nc.dram_tensor("wq", [D, DL], BF16, kind="ExternalInput")
    wk_d = nc.dram_tensor("wk", [D, DL], BF16, kind="ExternalInput")
    wv_d = nc.dram_tensor("wv", [D, DL], BF16, kind="ExternalInput")
    wo_d = nc.dram_tensor("wout", [D, DL], BF16, kind="ExternalInput")
    bq_d = nc.dram_tensor("bq", [P, NMT], F32, kind="ExternalInput")
    bk_d = nc.dram_tensor("bk", [P, NMT], F32, kind="ExternalInput")
    bv_d = nc.dram_tensor("bv", [P, DL], F32, kind="ExternalInput")
    bo_d = nc.dram_tensor("bo", [P, NMT], F32, kind="ExternalInput")
    tri_d = nc.dram_tensor("tri", [P, P], BF16, kind="ExternalInput")
    ones_d = nc.dram_tensor("ones", [P, 32], BF16, kind="ExternalInput")
    outT_d = nc.dram_tensor("outT", [DL, T], F32, kind="ExternalOutput")

    # chunks 0-1 share ONE AllGather (issued after chunk 1's normalize):
    # fewer collectives on the serial CC stream.
    ag_in01 = nc.dram_tensor("ag_in01", [DL, 1024], BF16)
    ag_in2 = nc.dram_tensor("ag_in2", [DL, 512], BF16)
    ag_in34 = nc.dram_tensor("ag_in34", [DL, 512], BF16)
    ag_in = [ag_in01, ag_in01, ag_in2, ag_in34, ag_in34]
    ag_out01 = nc.dram_tensor("ag_out01", [D, 1024], BF16)
    ag_out2 = nc.dram_tensor("ag_out2", [D, 512], BF16)
    ag_out34 = nc.dram_tensor("ag_out34", [D, 512], BF16)
    ag_out = [ag_out01, ag_out01, ag_out2, ag_out34, ag_out34]
    ag_warm_in = nc.dram_tensor("ag_warm_in", [32, 16], BF16)
    ag_warm_out = nc.dram_tensor("ag_warm_out", [128, 16], BF16)

    with tile.TileContext(nc) as tc:
        with (
            tc.tile_pool(name="const", bufs=1) as const,
            tc.tile_pool(name="expst", bufs=4) as expst_pool,
            tc.tile_pool(name="attn", bufs=2) as attn_pool,
            tc.tile_pool(name="agf", bufs=3) as agf_pool,
            tc.tile_pool(name="outsb", bufs=2) as out_pool,
            tc.tile_pool(name="recip", bufs=2) as recip_pool,
            tc.tile_pool(name="ps_s", bufs=2, space="PSUM") as ps_s_pool,
            tc.tile_pool(name="ps_pv", bufs=2, space="PSUM") as ps_pv_pool,
            tc.tile_pool(name="ps_sum", bufs=1, space="PSUM") as ps_sum_pool,
            tc.tile_pool(name="ps_mm", bufs=1, space="PSUM") as ps_mm_pool,
        ):
            # ---- constant loads, ordered by first use -------------------
            xt_view = xt_d[:].rearrange("(s p) t -> p s t", p=P)
            wq_sb = const.tile([P, FS, DL], BF16)
            nc.sync.dma_start(wq_sb[:], wq_d[:].rearrange("(s p) n -> p s n", p=P))
            xt_tc = [const.tile([P, FS, 512], BF16, tag=f"xt{i}", name=f"xt{i}")
                     for i in range(NTC)]
            # two half DMAs (4 slices each): big per-partition lines (4KB)
            # for full HBM BW, while qk half-groups can start on the first
            nc.sync.dma_start(xt_tc[0][:, 0:4, :], xt_view[:, 0:4, 0:512])
            nc.sync.dma_start(xt_tc[0][:, 4:8, :], xt_view[:, 4:8, 0:512])
            wk_sb = const.tile([P, FS, DL], BF16)
            nc.sync.dma_start(wk_sb[:], wk_d[:].rearrange("(s p) n -> p s n", p=P))
            bq_sb = const.tile([P, NMT], F32)
            nc.sync.dma_start(bq_sb[:], bq_d[:])
            bk_sb = const.tile([P, NMT], F32)
            nc.sync.dma_start(bk_sb[:], bk_d[:])
            wv_sb = const.tile([P, FS, DL], BF16)
            nc.sync.dma_start(wv_sb[:], wv_d[:].rearrange("(s p) n -> p s n", p=P))
            bv_sb = const.tile([P, DL], F32)
            nc.sync.dma_start(bv_sb[:], bv_d[:])
            tri_sb = const.tile([P, P], BF16)
            nc.sync.dma_start(tri_sb[:], tri_d[:])
            ones_sb = const.tile([P, 32], BF16)
            nc.sync.dma_start(ones_sb[:], ones_d[:])
            zb = const.tile([P, 1], F32)
            nc.gpsimd.memset(zb[:], 0.0)
            # preload the ACT exp table (1.3us) during the DMA window
            dummy_sb = const.tile([P, 1], F32)
            nc.scalar.activation(dummy_sb[:], zb[:],
                                 mybir.ActivationFunctionType.Exp,
                                 bias=zb[:], scale=0.125)
            for tcidx in range(1, NTC):
                nc.sync.dma_start(xt_tc[tcidx][:],
                                  xt_view[:, :, 512 * tcidx:512 * tcidx + 512])
            wo_sb = const.tile([P, FS, DL], BF16)
            nc.sync.dma_start(wo_sb[:], wo_d[:].rearrange("(s p) n -> p s n", p=P))
            bo_sb = const.tile([P, NMT], F32)
            nc.sync.dma_start(bo_sb[:], bo_d[:])

            qT_tc = [const.tile([P, NMT, 512], BF16, tag=f"qT{i}", name=f"qT{i}") for i in range(NTC)]
            kT_tc = [const.tile([P, NMT, 512], BF16, tag=f"kT{i}", name=f"kT{i}") for i in range(NTC)]
            v_tc = [const.tile([P, 4, DL], BF16, tag=f"v{i}", name=f"v{i}") for i in range(NTC)]

            # ---- qkv emission: full groups (head) and half groups (fillers)
            def qk_emit(tcx, dst, w_sb, b_sb, mt, pool=None):
                """Returns two closures, each 4 matmuls; epilogue on the
                2nd. cols per half: 2048 (0.85us at 2.4GHz)."""
                xt = xt_tc[tcx]
                mmtile = (lambda: ps_mm_pool.tile([P, 512], F32, tag="mm",
                                                  name="mmps")) \
                    if pool is None else pool
                state = {}

                def half(h):
                    def emit(after=None):
                        if h == 0:
                            state["ps"] = mmtile()
                        ps = state["ps"]
                        for s in range(4 * h, 4 * h + 4):
                            mm = nc.tensor.matmul(
                                ps[:], w_sb[:, s, P * mt:P * mt + P],
                                xt[:, s, :],
                                start=(s == 0), stop=(s == FS - 1))
                            if after is not None and s == 4 * h:
                                tile.add_dep_helper(mm.ins, after, sync=False,
                                                    reason="filler order")
                        if h == 1:
                            nc.vector.tensor_scalar_add(
                                dst[:, mt, :], ps[:], b_sb[:, mt:mt + 1])
                    return emit
                return [half(0), half(1)]

            def v_emit(tcx, tt, vpool=None):
                """Two closures of 4 matmuls each (1024 cols per half).
                Consecutive v groups use alternating halves of the psum
                bank so tt+1's matmuls don't wait tt's epilogue."""
                xt = xt_tc[tcx]
                vtile = (lambda: ps_mm_pool.tile([P, 512], F32, tag="mm",
                                                 name="mmps")) \
                    if vpool is None else vpool
                lo = DL * (tt % 2)
                state = {}

                def half(h):
                    def emit(after=None):
                        if h == 0:
                            state["ps"] = vtile()
                        ps = state["ps"]
                        for s in range(4 * h, 4 * h + 4):
                            mm = nc.tensor.matmul(
                                ps[:, lo:lo + DL], xt[:, s, P * tt:P * tt + P],
                                wv_sb[:, s, :],
                                start=(s == 0), stop=(s == FS - 1))
                            if after is not None and s == 4 * h:
                                tile.add_dep_helper(mm.ins, after, sync=False,
                                                    reason="filler order")
                        if h == 1:
                            nc.vector.tensor_add(
                                v_tc[tcx][:, tt, :], ps[:, lo:lo + DL], bv_sb[:])
                    return emit
                return [half(0), half(1)]

            # filler queue entries: (cols, closure, pin)
            Q_COLS, V_COLS = 2048, 1024

            def q_halves(tcx):
                out = []
                for mt in range(NMT):
                    for f in qk_emit(tcx, qT_tc[tcx], wq_sb, bq_sb, mt):
                        out.append((Q_COLS, f, False))
                return out

            def k_halves(tcx):
                out = []
                for mt in range(NMT):
                    for f in qk_emit(tcx, kT_tc[tcx], wk_sb, bk_sb, mt):
                        out.append((Q_COLS, f, False))
                return out

            def v_halves(tcx):
                out = []
                for tt in range(4):
                    for f in v_emit(tcx, tt):
                        out.append((V_COLS, f, False))
                return out

            def attention_chunk(ci, fillers=(), flush=False):
                """Attention for query chunk ci; returns the last PV matmul
                instruction (for pinning tail work behind it)."""
                q_lo, qlen = QCHUNKS[ci]
                q_tck, q_off0 = q_lo // 512, q_lo % 512
                nkk = (q_lo + qlen) // P
                fillers = list(fillers)
                ps_pv = [ps_pv_pool.tile([P, 512], F32, tag="pv",
                                         name=f"pv{ci}_{i}") for i in range(2)]
                ps_sum = ps_sum_pool.tile([P, 512], F32)
                last_mm = [None]

                def geom(kk):
                    tck, m = kk // 4, kk % 4
                    d = P * kk - q_lo
                    if d < 0:
                        return tck, m, False, 0, qlen
                    return tck, m, True, d, qlen - d

                # narrow chunks (q_len <= 128): all 4 heads' S fit in ONE
                # psum tile half-pair -> one exp instruction per key tile
                # instead of two (saves the 352-cycle ACT issue overhead).
                narrow = qlen <= 128

                def eslice(expst, h, n):
                    if narrow:
                        return expst[:, h % 2, qlen * (h // 2):
                                     qlen * (h // 2) + n]
                    return expst[:, h, 0:n]

                def emit_s_exp(kk):
                    """S^T in two 2-head halves, each its own 2-bank psum
                    tile (pool bufs=2) so S(g+1) issues while ACT still
                    reads exp(g)'s input."""
                    tck, m, diag, off, W = geom(kk)
                    expst = expst_pool.tile([P, 4, 512], BF16, tag="expst",
                                            name=f"expst{ci}_{kk}")
                    if narrow:
                        ps_s = ps_s_pool.tile([P, 2, 512], F32, tag="s",
                                              name=f"s{ci}_{kk}")
                        for h in range(4):
                            mt, rp = h // 2, 64 * (h % 2)
                            mm = nc.tensor.matmul(
                                ps_s[:, h % 2, qlen * (h // 2):
                                     qlen * (h // 2) + W],
                                kT_tc[tck][rp:rp + 64, mt, P * m:P * m + P],
                                qT_tc[q_tck][rp:rp + 64, mt,
                                             q_off0 + off:q_off0 + off + W],
                                start=True, stop=True)
                            last_mm[0] = mm.ins
                        nc.scalar.activation(
                            expst[:, 0:2, 0:2 * qlen],
                            ps_s[:, 0:2, 0:2 * qlen],
                            mybir.ActivationFunctionType.Exp,
                            bias=zb[:], scale=0.125)
                    else:
                        for hp in range(2):
                            ps_s = ps_s_pool.tile([P, 2, 512], F32, tag="s",
                                                  name=f"s{ci}_{kk}_{hp}")
                            for hh in range(2):
                                h = 2 * hp + hh
                                mt, rp = h // 2, 64 * (h % 2)
                                mm = nc.tensor.matmul(
                                    ps_s[:, hh, 0:W],
                                    kT_tc[tck][rp:rp + 64, mt,
                                               P * m:P * m + P],
                                    qT_tc[q_tck][rp:rp + 64, mt,
                                                 q_off0 + off:q_off0 + off + W],
                                    start=True, stop=True)
                                last_mm[0] = mm.ins
                            nc.scalar.activation(
                                expst[:, 2 * hp:2 * hp + 2, 0:W],
                                ps_s[:, 0:2, 0:W],
                                mybir.ActivationFunctionType.Exp,
                                bias=zb[:], scale=0.125)
                    if diag:
                        # SBUF-only bf16 muls -> idle GpSimd, keeping DVE
                        # free for the psum-slot-releasing epilogues
                        for h in range(4):
                            nc.gpsimd.tensor_mul(
                                eslice(expst, h, P),
                                eslice(expst, h, P), tri_sb[:])
                    return expst

                def emit_pv_sums(kk, expst):
                    tck, m, diag, off, W = geom(kk)
                    # PV^T accumulation (V stationary, exp moving), 2 heads/slot
                    for hp in range(2):
                        for hh in range(2):
                            h = 2 * hp + hh
                            mm = nc.tensor.matmul(
                                ps_pv[hp][64 * hh:64 * hh + 64, off:off + W],
                                v_tc[tck][:, m, 64 * h:64 * h + 64],
                                eslice(expst, h, W),
                                start=(kk == 0), stop=(kk == nkk - 1))
                            last_mm[0] = mm.ins
                    # softmax denominators: ones-matmuls (M=32 so each head's
                    # sum lands replicated on 32 partitions), 4 heads packed
                    # by 32-aligned column groups (partitions 32h..32h+31)
                    for h in range(4):
                        nc.tensor.matmul(
                            ps_sum[32 * h:32 * h + 32, off:off + W],
                            ones_sb[:, 0:32],
                            eslice(expst, h, W),
                            start=(kk == 0), stop=(kk == nkk - 1),
                            tile_position=(0, 32 * h))

                def pop_fillers(slots_left):
                    # column budget per key-tile slot so the next S pair
                    # (which gates the next exp) is never far back in the
                    # PE FIFO.  Looser budget when the queue is backlogged.
                    cols_left = sum(e[0] for e in fillers)
                    budget = 2048 if cols_left <= 2048 * slots_left else 4096
                    used = 0
                    while fillers and (used == 0
                                       or used + fillers[0][0] <= budget):
                        cols, f, pin = fillers.pop(0)
                        used += cols
                        f(last_mm[0] if pin else None)

                # Software-pipelined emission: S+exp for kk+1 go into the
                # engine queues BEFORE PV/sums for kk; fillers sit between
                # S(kk+1) and PV(kk) so they can only delay PV accumulation
                # (slack) and never the S->exp chain.
                expst_prev = emit_s_exp(0)
                for kk in range(1, nkk):
                    expst_cur = emit_s_exp(kk)
                    pop_fillers(nkk - kk)
                    emit_pv_sums(kk - 1, expst_prev)
                    expst_prev = expst_cur
                emit_pv_sums(nkk - 1, expst_prev)
                last_pv = last_mm[0]
                # leftovers carry over to the next chunk's queue unless this
                # is the last filler-bearing chunk
                while flush and fillers:
                    _, f, pin = fillers.pop(0)
                    f(last_mm[0] if pin else None)
                # normalize + stage for the AllGather. Chain the muls with
                # no-sync deps so hp0 finishes (and releases its PV psum
                # slot for the next chunk) before hp1 starts.
                recip = recip_pool.tile([P, 512], F32)
                nc.vector.reciprocal_approx_fast(recip[:, 0:qlen],
                                                 ps_sum[:, 0:qlen])
                prev_mul = None
                for hp in range(2):
                    attn = attn_pool.tile([P, 512], BF16)
                    for hh in range(2):
                        h = 2 * hp + hh
                        for half in range(2):
                            lo = 64 * hh + 32 * half
                            mul = nc.vector.tensor_mul(
                                attn[lo:lo + 32, 0:qlen],
                                ps_pv[hp][lo:lo + 32, 0:qlen],
                                recip[32 * h:32 * h + 32, 0:qlen])
                            if prev_mul is not None:
                                tile.add_dep_helper(
                                    mul.ins, prev_mul.ins, sync=False,
                                    reason="normalize order hp0-first")
                            prev_mul = mul
                    nc.sync.dma_start(
                        ag_in[ci][P * hp:P * hp + P,
                                  q_lo - QCHUNKS[AG_HEAD[ci]][0]:
                                  q_lo - QCHUNKS[AG_HEAD[ci]][0] + qlen],
                        attn[:, 0:qlen])
                if ci == AG_TAIL[ci]:
                    ag_chunk(ci)
                return last_pv, fillers

            # AG groups: chunks 0-1 share one collective (triggered after
            # chunk 1's normalize), chunk 2 its own, chunks 3-4 share one.
            # Fewer ops on the serial CC stream: each trigger waits the
            # previous op's completion (+~7us), so ops are expensive.
            AG_HEAD = [0, 0, 2, 3, 3]
            AG_TAIL = [1, 1, 2, 4, 4]
            agf_tiles = {}

            def ag_chunk(ci):
                glen = sum(QCHUNKS[c][1] for c in range(NQC)
                           if AG_HEAD[c] == AG_HEAD[ci])
                nc.gpsimd.collective_compute(
                    "AllGather", mybir.AluOpType.bypass,
                    replica_groups=GROUPS,
                    ins=[ag_in[ci][:]], outs=[ag_out[ci][:]])
                if ci <= 1:  # one-shot big tile; const pool (bufs=1)
                    agf = const.tile([P, FS, 1024], BF16, tag="agf01",
                                     name="agf01")
                else:
                    agf = agf_pool.tile([P, FS, 512], BF16, name=f"agf{ci}")
                # two half DMAs: proj can start on the first half while the
                # second lands, without paying 8 separate DMA-issue costs
                agv = ag_out[ci][:].rearrange("(s p) t -> p s t", p=P)
                nc.sync.dma_start(agf[:, 0:4, 0:glen], agv[:, 0:4, :])
                nc.sync.dma_start(agf[:, 4:8, 0:glen], agv[:, 4:8, :])
                agf_tiles[AG_HEAD[ci]] = agf

            def proj_groups(ci, pool=None):
                q_lo, qlen = QCHUNKS[ci]
                agoff = q_lo - QCHUNKS[AG_HEAD[ci]][0]
                mmtile = (lambda: ps_mm_pool.tile([P, 512], F32, tag="mm",
                                                  name="projmm")) \
                    if pool is None else pool

                def group(mt):
                    def emit(after=None):
                        agf = agf_tiles[AG_HEAD[ci]]
                        ps = mmtile()
                        for s in range(FS):
                            mm = nc.tensor.matmul(
                                ps[:, 0:qlen],
                                wo_sb[:, s, P * mt:P * mt + P],
                                agf[:, s, agoff:agoff + qlen],
                                start=(s == 0), stop=(s == FS - 1))
                            if after is not None and s == 0:
                                tile.add_dep_helper(mm.ins, after, sync=False,
                                                    reason="proj after attn")
                        osb = out_pool.tile([P, 512], F32)
                        if mt == 1:  # alternate epilogue engine (ACT idle in tail)
                            nc.scalar.add(osb[:, 0:qlen], ps[:, 0:qlen],
                                          bo_sb[:, mt:mt + 1])
                        else:
                            nc.vector.tensor_scalar_add(
                                osb[:, 0:qlen], ps[:, 0:qlen],
                                bo_sb[:, mt:mt + 1])
                        nc.sync.dma_start(
                            outT_d[P * mt:P * mt + P, q_lo:q_lo + qlen],
                            osb[:, 0:qlen])
                    return emit
                return [group(mt) for mt in range(NMT)]

            # ---- emission schedule --------------------------------------
            # PE warmup sized to the input-DMA window (~3us): HAM needs
            # ~3.4us of activity to un-throttle; oversizing delays qk0.
            warm_sb = const.tile([P, 256], BF16)
            nc.gpsimd.memset(warm_sb[:], 0.0)
            ps_w = ps_mm_pool.tile([P, 512], F32, tag="mm")
            for _ in range(14):
                nc.tensor.matmul(ps_w[:, 0:256], warm_sb[:, 0:P], warm_sb[:],
                                 start=True, stop=True)
            # dummy collective: the collectives stack pays its one-time
            # init + entry barrier (~50us) during the compute head
            nc.gpsimd.collective_compute(
                "AllGather", mybir.AluOpType.bypass, replica_groups=GROUPS,
                ins=[ag_warm_in[:]], outs=[ag_warm_out[:]])

            # chunk-0 q/k double-buffer through the ps_s slots and v(0)
            # through the (still idle) ps_pv slots -- two independent psum
            # chains run concurrently before attention(0) starts.
            spool = lambda: ps_s_pool.tile(
                [P, 2, 512], F32, tag="s", name="qkv0mm")[:, 0, :]
            vpool = lambda: ps_pv_pool.tile([P, 512], F32, tag="pv",
                                            name="qkv0v")
            for mt in range(NMT):
                for f in qk_emit(0, qT_tc[0], wq_sb, bq_sb, mt, pool=spool):
                    f()
            for mt in range(NMT):
                for f in qk_emit(0, kT_tc[0], wk_sb, bk_sb, mt, pool=spool):
                    f()
            for f in v_emit(0, 0, vpool=vpool):
                f()

            # per-chunk filler queues (half-group granularity), paced so
            # producers finish a few key-tiles before their consumers;
            # leftovers carry into the next chunk's queue.  proj(0)/proj(1)
            # ride as pinned fillers in c3/c4 (their AG completes mid-c3)
            # to keep the PE warm there; proj(2..4) form the tail backlog
            # that hides the last AllGather.
            f_c0 = [(V_COLS, f, False) for tt in (1, 2, 3)
                    for f in v_emit(0, tt)] + q_halves(1)
            _, rest = attention_chunk(0, f_c0)
            f_c1 = rest + k_halves(1) + v_halves(1) + q_halves(2)
            _, rest = attention_chunk(1, f_c1)
            f_c2 = rest + k_halves(2) + v_halves(2) + q_halves(3)
            _, rest = attention_chunk(2, f_c2)
            f_c3 = rest + k_halves(3) + v_halves(3) \
                + [(4096, g, True) for g in proj_groups(0)]
            _, rest = attention_chunk(3, f_c3)
            f_c4 = rest + [(4096, g, True) for g in proj_groups(1)]
            last_pv, _ = attention_chunk(4, f_c4, flush=True)
            # tail: proj(2..4) deferred behind the last AllGather trigger;
            # they run from the (now free) ps_s psum banks so consecutive
            # groups never serialize on a single-bank WAR.
            def tailpool():
                ps = ps_s_pool.tile([P, 2, 512], F32, tag="s",
                                    name="projps")
                return ps[:, 0, :]
            for ci in (2, 3, 4):
                for g in proj_groups(ci, pool=tailpool):
                    g(last_pv)

    nc.compile()
    return nc


_NC_CACHE = None


def _get_nc():
    global _NC_CACHE
    if _NC_CACHE is None:
        _NC_CACHE = build_bass()
    return _NC_CACHE


def _make_in_maps(x, Wqkv, bqkv, Wout, bout):
    bf16 = ml_dtypes.bfloat16
    in_maps = []
    for c in range(NCORES):
        b, g = c // 4, c % 4
        cs = DL * g  # column/dim slice start for this core's heads
        im = {
            "xt": np.ascontiguousarray(x[b].T).astype(bf16),
            "wq": np.ascontiguousarray(Wqkv[:, cs:cs + DL]).astype(bf16),
            "wk": np.ascontiguousarray(Wqkv[:, D + cs:D + cs + DL]).astype(bf16),
            "wv": np.ascontiguousarray(Wqkv[:, 2 * D + cs:2 * D + cs + DL]).astype(bf16),
            "wout": np.ascontiguousarray(Wout[:, cs:cs + DL]).astype(bf16),
            "bq": np.ascontiguousarray(
                bqkv[cs:cs + DL].reshape(NMT, P).T).astype(np.float32),
            "bk": np.ascontiguousarray(
                bqkv[D + cs:D + cs + DL].reshape(NMT, P).T).astype(np.float32),
            "bv": np.ascontiguousarray(np.broadcast_to(
                bqkv[2 * D + cs:2 * D + cs + DL].reshape(1, DL),
                (P, DL))).astype(np.float32),
            "bo": np.ascontiguousarray(
                bout[cs:cs + DL].reshape(NMT, P).T).astype(np.float32),
            "tri": np.triu(np.ones((P, P))).astype(bf16),
            "ones": np.ones((P, 32), dtype=bf16),
        }
        in_maps.append(im)
    return in_maps


def _run(inputs, trace=False, tmpdir=None):
    nc = _get_nc()
    in_maps = _make_in_maps(**inputs)
    res = bass_utils.run_bass_kernel_spmd(
        nc, in_maps, core_ids=list(range(NCORES)), trace=trace, tmpdir=tmpdir)
    out = np.empty((B, T, D), dtype=np.float32)
    for c in range(NCORES):
        b, g = c // 4, c % 4
        out[b, :, DL * g:DL * g + DL] = res.results[c]["outT"].T
    return out, res


def kernel(x, Wqkv, bqkv, Wout, bout):
    out, _ = _run(dict(x=np.asarray(x, dtype=np.float32),
                       Wqkv=np.asarray(Wqkv, dtype=np.float32),
                       bqkv=np.asarray(bqkv, dtype=np.float32),
                       Wout=np.asarray(Wout, dtype=np.float32),
                       bout=np.asarray(bout, dtype=np.float32)))
    return out


# revision 19
# speedup vs baseline: 1.0645x; 1.0234x over previous
"""Causal self-attention Trainium2 kernel (8 NeuronCores, SPMD).

Problem: B=2, T=2048, D=1024, H=16 heads (head_dim 64), fp32 I/O.
    qkv = x @ Wqkv + bqkv ; per-head causal softmax(q k^T / 8) @ v ; out @ Wout + bout

Sharding: 2 batch groups x 4 cores. Core c: batch b=c//4, head group g=c%4
(heads 4g..4g+3, i.e. D-slice [256g, 256g+256)), and out-proj column slice
[256g, 256g+256). Attention outputs are AllGathered (bf16) within each
4-core batch group per query chunk; out-proj is column-sharded so the
final output needs no reduction -- each core returns a [256, 2048] slice
(transposed) which the host reassembles.

Engine budget (per core): ACT exp ~8.9M elems ~= 90us is the steady-state
bottleneck; PE ~220K matmul-slot columns ~= 95-115us; the serial CC stream
(entry barrier ~50us one-time + ~15-25us per AllGather) is the tail
bottleneck.  The schedule therefore optimizes (a) time-to-first-exp,
(b) exp-stream density, (c) hiding the last AllGather behind deferred
out-proj work.

Key scheduling decisions (vs a naive emission):
  - DMA order puts wq + x-chunk0 + wk first so qk(0) can start ~10us in;
    a dummy exp preloads the ACT table (1.3us) during the DMA window;
    PE warmup is ~14 N=256 matmuls sized to the DMA window (HAM un-throttle
    needs ~3.4us of activity, and oversizing delays qk0 in the PE FIFO).
  - qkv groups for chunk c+1 are emitted as 4-matmul HALF-groups paced
    into attention(c)'s per-key-tile slots by a column budget, so the
    S(kk+1) matmuls (which gate the next exp) never sit behind >1us of
    filler in the PE FIFO.
  - PV psum is copied to SBUF (DVE) right after the last PV matmul and
    the softmax normalize runs from SBUF, so the 2 PV psum banks free in
    ~1us instead of ~5us and the next chunk's PV never head-of-line
    blocks the S stream at a chunk boundary.
  - ALL out-proj groups are deferred until after the LAST AllGather
    trigger: proj(0..3) (~13us of PE work, data-ready) then fill the
    ~16us latency of AG(4); only proj(4) (0.9us) is exposed.
  - The last 512 tokens are split 384+128 and the 128-wide chunk uses the
    "narrow" path: all 4 heads' S in one psum tile -> ONE exp per key
    tile (halves ACT instruction overhead where ACT overhead dominates).
  - A dummy 1KB AllGather is triggered first thing: the collectives stack
    pays its ~50us one-time init/entry-barrier during the compute head
    instead of on the first real AllGather.
"""

import numpy as np
import ml_dtypes

import concourse.bass as bass
import concourse.tile as tile
from concourse import bacc, bass_utils, mybir

BF16 = mybir.dt.bfloat16
F32 = mybir.dt.float32

B, T, D, H = 2, 2048, 1024, 16
HD = D // H  # 64
NCORES = 8
GROUPS = [[0, 1, 2, 3], [4, 5, 6, 7]]
P = 128  # partitions
FS = D // P  # 8 feature slices
NTC = T // 512  # 4 key/token chunks (k/v tiling; fixed)
DL = 256  # local d (4 heads * 64)
NMT = DL // P  # 2 stationary M-tiles

# query chunks: (q_lo, q_len); last 512 split 384+128 to shrink the tail
QCHUNKS = [(0, 512), (512, 512), (1024, 512), (1536, 384), (1920, 128)]
NQC = len(QCHUNKS)


def build_bass():
    nc = bacc.Bacc("TRN2", target_bir_lowering=False, debug=False,
                   num_devices=NCORES)

    xt_d = nc.dram_tensor("xt", [D, T], BF16, kind="ExternalInput")
    wq_d = nc.dram_tensor("wq", [D, DL], BF16, kind="ExternalInput")
    wk_d = nc.dram_tensor("wk", [D, DL], BF16, kind="ExternalInput")
    wv_d = nc.dram_tensor("wv", [D, DL], BF16, kind="ExternalInput")
    wo_d = nc.dram_tensor("wout", [D, DL], BF16, kind="ExternalInput")
    bq_d = nc.dram_tensor("bq", [P, NMT], F32, kind="ExternalInput")
    bk_d = nc.dram_tensor("bk", [P, NMT], F32, kind="ExternalInput")
    bv_d = nc.dram_tensor("bv", [P, DL], F32, kind="ExternalInput")
    bo_d = nc.dram_tensor("bo", [P, NMT], F32, kind="ExternalInput")
    tri_d = nc.dram_tensor("tri", [P, P], BF16, kind="ExternalInput")
    ones_d = nc.dram_tensor("ones", [P, 32], BF16, kind="ExternalInput")
    outT_d = nc.dram_tensor("outT", [DL, T], F32, kind="ExternalOutput")

    # chunks 0-1 share ONE AllGather (issued after chunk 1's normalize):
    # fewer collectives on the serial CC stream.
    ag_in01 = nc.dram_tensor("ag_in01", [DL, 1024], BF16)
    ag_in2 = nc.dram_tensor("ag_in2", [DL, 512], BF16)
    ag_in34 = nc.dram_tensor("ag_in34", [DL, 512], BF16)
    ag_in = [ag_in01, ag_in01, ag_in2, ag_in34, ag_in34]
    ag_out01 = nc.dram_tensor("ag_out01", [D, 1024], BF16)
    ag_out2 = nc.dram_tensor("ag_out2", [D, 512], BF16)
    ag_out34 = nc.dram_tensor("ag_out34", [D, 512], BF16)
    ag_out = [ag_out01, ag_out01, ag_out2, ag_out34, ag_out34]
    ag_warm_in = nc.dram_tensor("ag_warm_in", [32, 16], BF16)
    ag_warm_out = nc.dram_tensor("ag_warm_out", [128, 16], BF16)

    with tile.TileContext(nc) as tc:
        with (
            tc.tile_pool(name="const", bufs=1) as const,
            tc.tile_pool(name="expst", bufs=4) as expst_pool,
            tc.tile_pool(name="attn", bufs=3) as attn_pool,
            tc.tile_pool(name="agf", bufs=3) as agf_pool,
            tc.tile_pool(name="outsb", bufs=4) as out_pool,
            tc.tile_pool(name="recip", bufs=2) as recip_pool,
            tc.tile_pool(name="ps_s", bufs=2, space="PSUM") as ps_s_pool,
            tc.tile_pool(name="ps_pv", bufs=2, space="PSUM") as ps_pv_pool,
            tc.tile_pool(name="ps_sum", bufs=1, space="PSUM") as ps_sum_pool,
            tc.tile_pool(name="ps_mm", bufs=1, space="PSUM") as ps_mm_pool,
        ):
            # ---- constant loads, ordered by first use -------------------
            xt_view = xt_d[:].rearrange("(s p) t -> p s t", p=P)
            wq_sb = const.tile([P, FS, DL], BF16)
            nc.sync.dma_start(wq_sb[:], wq_d[:].rearrange("(s p) n -> p s n", p=P))
            xt_tc = [const.tile([P, FS, 512], BF16, tag=f"xt{i}", name=f"xt{i}")
                     for i in range(NTC)]
            # two half DMAs (4 slices each): big per-partition lines (4KB)
            # for full HBM BW, while qk half-groups can start on the first
            nc.sync.dma_start(xt_tc[0][:, 0:4, :], xt_view[:, 0:4, 0:512])
            nc.sync.dma_start(xt_tc[0][:, 4:8, :], xt_view[:, 4:8, 0:512])
            wk_sb = const.tile([P, FS, DL], BF16)
            nc.sync.dma_start(wk_sb[:], wk_d[:].rearrange("(s p) n -> p s n", p=P))
            bq_sb = const.tile([P, NMT], F32)
            nc.sync.dma_start(bq_sb[:], bq_d[:])
            bk_sb = const.tile([P, NMT], F32)
            nc.sync.dma_start(bk_sb[:], bk_d[:])
            wv_sb = const.tile([P, FS, DL], BF16)
            nc.sync.dma_start(wv_sb[:], wv_d[:].rearrange("(s p) n -> p s n", p=P))
            bv_sb = const.tile([P, DL], F32)
            nc.sync.dma_start(bv_sb[:], bv_d[:])
            tri_sb = const.tile([P, P], BF16)
            nc.sync.dma_start(tri_sb[:], tri_d[:])
            ones_sb = const.tile([P, 32], BF16)
            nc.sync.dma_start(ones_sb[:], ones_d[:])
            zb = const.tile([P, 1], F32)
            nc.gpsimd.memset(zb[:], 0.0)
            # preload the ACT exp table (1.3us) during the DMA window
            dummy_sb = const.tile([P, 1], F32)
            nc.scalar.activation(dummy_sb[:], zb[:],
                                 mybir.ActivationFunctionType.Exp,
                                 bias=zb[:], scale=0.125)
            for tcidx in range(1, NTC):
                nc.sync.dma_start(xt_tc[tcidx][:],
                                  xt_view[:, :, 512 * tcidx:512 * tcidx + 512])
            wo_sb = const.tile([P, FS, DL], BF16)
            nc.sync.dma_start(wo_sb[:], wo_d[:].rearrange("(s p) n -> p s n", p=P))
            bo_sb = const.tile([P, NMT], F32)
            nc.sync.dma_start(bo_sb[:], bo_d[:])

            qT_tc = [const.tile([P, NMT, 512], BF16, tag=f"qT{i}", name=f"qT{i}") for i in range(NTC)]
            kT_tc = [const.tile([P, NMT, 512], BF16, tag=f"kT{i}", name=f"kT{i}") for i in range(NTC)]
            v_tc = [const.tile([P, 4, DL], BF16, tag=f"v{i}", name=f"v{i}") for i in range(NTC)]

            # ---- qkv emission: full groups (head) and half groups (fillers)
            def qk_emit(tcx, dst, w_sb, b_sb, mt, pool=None):
                """Returns two closures, each 4 matmuls; epilogue on the
                2nd. cols per half: 2048 (0.85us at 2.4GHz)."""
                xt = xt_tc[tcx]
                mmtile = (lambda: ps_mm_pool.tile([P, 512], F32, tag="mm",
                                                  name="mmps")) \
                    if pool is None else pool
                state = {}

                def half(h):
                    def emit(after=None):
                        if h == 0:
                            state["ps"] = mmtile()
                        ps = state["ps"]
                        for s in range(4 * h, 4 * h + 4):
                            mm = nc.tensor.matmul(
                                ps[:], w_sb[:, s, P * mt:P * mt + P],
                                xt[:, s, :],
                                start=(s == 0), stop=(s == FS - 1))
                            if after is not None and s == 4 * h:
                                tile.add_dep_helper(mm.ins, after, sync=False,
                                                    reason="filler order")
                        if h == 1:
                            nc.vector.tensor_scalar_add(
                                dst[:, mt, :], ps[:], b_sb[:, mt:mt + 1])
                    return emit
                return [half(0), half(1)]

            def v_emit(tcx, tt, vpool=None):
                """Two closures of 4 matmuls each (1024 cols per half).
                Consecutive v groups use alternating halves of the psum
                bank so tt+1's matmuls don't wait tt's epilogue."""
                xt = xt_tc[tcx]
                vtile = (lambda: ps_mm_pool.tile([P, 512], F32, tag="mm",
                                                 name="mmps")) \
                    if vpool is None else vpool
                lo = DL * (tt % 2)
                state = {}

                def half(h):
                    def emit(after=None):
                        if h == 0:
                            state["ps"] = vtile()
                        ps = state["ps"]
                        for s in range(4 * h, 4 * h + 4):
                            mm = nc.tensor.matmul(
                                ps[:, lo:lo + DL], xt[:, s, P * tt:P * tt + P],
                                wv_sb[:, s, :],
                                start=(s == 0), stop=(s == FS - 1))
                            if after is not None and s == 4 * h:
                                tile.add_dep_helper(mm.ins, after, sync=False,
                                                    reason="filler order")
                        if h == 1:
                            nc.vector.tensor_add(
                                v_tc[tcx][:, tt, :], ps[:, lo:lo + DL], bv_sb[:])
                    return emit
                return [half(0), half(1)]

            # filler queue entries: (cols, closure, pin)
            Q_COLS, V_COLS = 2048, 1024

            def q_halves(tcx):
                out = []
                for mt in range(NMT):
                    for f in qk_emit(tcx, qT_tc[tcx], wq_sb, bq_sb, mt):
                        out.append((Q_COLS, f, False))
                return out

            def k_halves(tcx):
                out = []
                for mt in range(NMT):
                    for f in qk_emit(tcx, kT_tc[tcx], wk_sb, bk_sb, mt):
                        out.append((Q_COLS, f, False))
                return out

            def v_halves(tcx):
                out = []
                for tt in range(4):
                    for f in v_emit(tcx, tt):
                        out.append((V_COLS, f, False))
                return out

            def attention_chunk(ci, fillers=(), flush=False):
                """Attention for query chunk ci; returns the last PV matmul
                instruction (for pinning tail work behind it)."""
                q_lo, qlen = QCHUNKS[ci]
                q_tck, q_off0 = q_lo // 512, q_lo % 512
                nkk = (q_lo + qlen) // P
                fillers = list(fillers)
                ps_pv = [ps_pv_pool.tile([P, 512], F32, tag="pv",
                                         name=f"pv{ci}_{i}") for i in range(2)]
                ps_sum = ps_sum_pool.tile([P, 512], F32)
                last_mm = [None]

                def geom(kk):
                    tck, m = kk // 4, kk % 4
                    d = P * kk - q_lo
                    if d < 0:
                        return tck, m, False, 0, qlen
                    return tck, m, True, d, qlen - d

                # narrow chunks (q_len <= 128): all 4 heads' S fit in ONE
                # psum tile half-pair -> one exp instruction per key tile
                # instead of two (saves the 352-cycle ACT issue overhead).
                narrow = qlen <= 128

                def eslice(expst, h, n):
                    if narrow:
                        return expst[:, h % 2, qlen * (h // 2):
                                     qlen * (h // 2) + n]
                    return expst[:, h, 0:n]

                def emit_s_exp(kk):
                    """S^T in two 2-head halves, each its own 2-bank psum
                    tile (pool bufs=2) so S(g+1) issues while ACT still
                    reads exp(g)'s input."""
                    tck, m, diag, off, W = geom(kk)
                    expst = expst_pool.tile([P, 4, 512], BF16, tag="expst",
                                            name=f"expst{ci}_{kk}")
                    if narrow:
                        ps_s = ps_s_pool.tile([P, 2, 512], F32, tag="s",
                                              name=f"s{ci}_{kk}")
                        for h in range(4):
                            mt, rp = h // 2, 64 * (h % 2)
                            mm = nc.tensor.matmul(
                                ps_s[:, h % 2, qlen * (h // 2):
                                     qlen * (h // 2) + W],
                                kT_tc[tck][rp:rp + 64, mt, P * m:P * m + P],
                                qT_tc[q_tck][rp:rp + 64, mt,
                                             q_off0 + off:q_off0 + off + W],
                                start=True, stop=True)
                            last_mm[0] = mm.ins
                        nc.scalar.activation(
                            expst[:, 0:2, 0:2 * qlen],
                            ps_s[:, 0:2, 0:2 * qlen],
                            mybir.ActivationFunctionType.Exp,
                            bias=zb[:], scale=0.125)
                    else:
                        for hp in range(2):
                            ps_s = ps_s_pool.tile([P, 2, 512], F32, tag="s",
                                                  name=f"s{ci}_{kk}_{hp}")
                            for hh in range(2):
                                h = 2 * hp + hh
                                mt, rp = h // 2, 64 * (h % 2)
                                mm = nc.tensor.matmul(
                                    ps_s[:, hh, 0:W],
                                    kT_tc[tck][rp:rp + 64, mt,
                                               P * m:P * m + P],
                                    qT_tc[q_tck][rp:rp + 64, mt,
                                                 q_off0 + off:q_off0 + off + W],
                                    start=True, stop=True)
                                last_mm[0] = mm.ins
                            nc.scalar.activation(
                                expst[:, 2 * hp:2 * hp + 2, 0:W],
                                ps_s[:, 0:2, 0:W],
                                mybir.ActivationFunctionType.Exp,
                                bias=zb[:], scale=0.125)
                    if diag:
                        # SBUF-only bf16 muls -> idle GpSimd, keeping DVE
                        # free for the psum-slot-releasing epilogues
                        for h in range(4):
                            nc.gpsimd.tensor_mul(
                                eslice(expst, h, P),
                                eslice(expst, h, P), tri_sb[:])
                    return expst

                def emit_pv_sums(kk, expst):
                    tck, m, diag, off, W = geom(kk)
                    # PV^T accumulation (V stationary, exp moving), 2 heads/slot
                    for hp in range(2):
                        for hh in range(2):
                            h = 2 * hp + hh
                            mm = nc.tensor.matmul(
                                ps_pv[hp][64 * hh:64 * hh + 64, off:off + W],
                                v_tc[tck][:, m, 64 * h:64 * h + 64],
                                eslice(expst, h, W),
                                start=(kk == 0), stop=(kk == nkk - 1))
                            last_mm[0] = mm.ins
                    # softmax denominators: ones-matmuls (M=32 so each head's
                    # sum lands replicated on 32 partitions), 4 heads packed
                    # by 32-aligned column groups (partitions 32h..32h+31)
                    for h in range(4):
                        nc.tensor.matmul(
                            ps_sum[32 * h:32 * h + 32, off:off + W],
                            ones_sb[:, 0:32],
                            eslice(expst, h, W),
                            start=(kk == 0), stop=(kk == nkk - 1),
                            tile_position=(0, 32 * h))

                def pop_fillers(slots_left):
                    # column budget per key-tile slot so the next S pair
                    # (which gates the next exp) is never far back in the
                    # PE FIFO.  Looser budget when the queue is backlogged.
                    cols_left = sum(e[0] for e in fillers)
                    budget = 2048 if cols_left <= 2048 * slots_left else 4096
                    used = 0
                    while fillers and (used == 0
                                       or used + fillers[0][0] <= budget):
                        cols, f, pin = fillers.pop(0)
                        used += cols
                        f(last_mm[0] if pin else None)

                # Software-pipelined emission: S+exp for kk+1 go into the
                # engine queues BEFORE PV/sums for kk; fillers sit between
                # S(kk+1) and PV(kk) so they can only delay PV accumulation
                # (slack) and never the S->exp chain.
                expst_prev = emit_s_exp(0)
                for kk in range(1, nkk):
                    expst_cur = emit_s_exp(kk)
                    pop_fillers(nkk - kk)
                    emit_pv_sums(kk - 1, expst_prev)
                    expst_prev = expst_cur
                emit_pv_sums(nkk - 1, expst_prev)
                last_pv = last_mm[0]
                # leftovers carry over to the next chunk's queue unless this
                # is the last filler-bearing chunk
                while flush and fillers:
                    _, f, pin = fillers.pop(0)
                    f(last_mm[0] if pin else None)
                # normalize + stage for the AllGather. Chain the muls with
                # no-sync deps so hp0 finishes (and releases its PV psum
                # slot for the next chunk) before hp1 starts.
                recip = recip_pool.tile([P, 512], F32)
                nc.vector.reciprocal_approx_fast(recip[:, 0:qlen],
                                                 ps_sum[:, 0:qlen])
                prev_mul = None
                for hp in range(2):
                    attn = attn_pool.tile([P, 512], BF16)
                    for hh in range(2):
                        h = 2 * hp + hh
                        for half in range(2):
                            lo = 64 * hh + 32 * half
                            mul = nc.vector.tensor_mul(
                                attn[lo:lo + 32, 0:qlen],
                                ps_pv[hp][lo:lo + 32, 0:qlen],
                                recip[32 * h:32 * h + 32, 0:qlen])
                            if prev_mul is not None:
                                tile.add_dep_helper(
                                    mul.ins, prev_mul.ins, sync=False,
                                    reason="normalize order hp0-first")
                            prev_mul = mul
                    nc.sync.dma_start(
                        ag_in[ci][P * hp:P * hp + P,
                                  q_lo - QCHUNKS[AG_HEAD[ci]][0]:
                                  q_lo - QCHUNKS[AG_HEAD[ci]][0] + qlen],
                        attn[:, 0:qlen])
                if ci == AG_TAIL[ci]:
                    ag_chunk(ci)
                return last_pv, fillers

            # AG groups: chunks 0-1 share one collective (triggered after
            # chunk 1's normalize), chunk 2 its own, chunks 3-4 share one.
            # Fewer ops on the serial CC stream: each trigger waits the
            # previous op's completion (+~7us), so ops are expensive.
            AG_HEAD = [0, 0, 2, 3, 3]
            AG_TAIL = [1, 1, 2, 4, 4]
            agf_tiles = {}

            def ag_chunk(ci):
                glen = sum(QCHUNKS[c][1] for c in range(NQC)
                           if AG_HEAD[c] == AG_HEAD[ci])
                nc.gpsimd.collective_compute(
                    "AllGather", mybir.AluOpType.bypass,
                    replica_groups=GROUPS,
                    ins=[ag_in[ci][:]], outs=[ag_out[ci][:]])
                if ci <= 1:  # one-shot big tile; const pool (bufs=1)
                    agf = const.tile([P, FS, 1024], BF16, tag="agf01",
                                     name="agf01")
                else:
                    agf = agf_pool.tile([P, FS, 512], BF16, name=f"agf{ci}")
                # two half DMAs: proj can start on the first half while the
                # second lands, without paying 8 separate DMA-issue costs
                agv = ag_out[ci][:].rearrange("(s p) t -> p s t", p=P)
                nc.sync.dma_start(agf[:, 0:4, 0:glen], agv[:, 0:4, :])
                nc.sync.dma_start(agf[:, 4:8, 0:glen], agv[:, 4:8, :])
                agf_tiles[AG_HEAD[ci]] = agf

            def proj_groups(ci, pool=None):
                q_lo, qlen = QCHUNKS[ci]
                agoff = q_lo - QCHUNKS[AG_HEAD[ci]][0]
                mmtile = (lambda: ps_mm_pool.tile([P, 512], F32, tag="mm",
                                                  name="projmm")) \
                    if pool is None else pool

                def group(mt):
                    def emit(after=None):
                        agf = agf_tiles[AG_HEAD[ci]]
                        ps = mmtile()
                        for s in range(FS):
                            mm = nc.tensor.matmul(
                                ps[:, 0:qlen],
                                wo_sb[:, s, P * mt:P * mt + P],
                                agf[:, s, agoff:agoff + qlen],
                                start=(s == 0), stop=(s == FS - 1))
                            if after is not None and s == 0:
                                tile.add_dep_helper(mm.ins, after, sync=False,
                                                    reason="proj after attn")
                        osb = out_pool.tile([P, 512], F32)
                        if mt == 1:  # alternate epilogue engine (ACT idle in tail)
                            nc.scalar.add(osb[:, 0:qlen], ps[:, 0:qlen],
                                          bo_sb[:, mt:mt + 1])
                        else:
                            nc.vector.tensor_scalar_add(
                                osb[:, 0:qlen], ps[:, 0:qlen],
                                bo_sb[:, mt:mt + 1])
                        nc.sync.dma_start(
                            outT_d[P * mt:P * mt + P, q_lo:q_lo + qlen],
                            osb[:, 0:qlen])
                    return emit
                return [group(mt) for mt in range(NMT)]

            # ---- emission schedule --------------------------------------
            # PE warmup sized to the input-DMA window (~3us): HAM needs
            # ~3.4us of activity to un-throttle; oversizing delays qk0.
            warm_sb = const.tile([P, 512], BF16)
            nc.gpsimd.memset(warm_sb[:], 0.0)
            ps_w = ps_mm_pool.tile([P, 512], F32, tag="mm")
            for _ in range(11):
                nc.tensor.matmul(ps_w[:], warm_sb[:, 0:P], warm_sb[:],
                                 start=True, stop=True)
            # dummy collective: the collectives stack pays its one-time
            # init + entry barrier (~50us) during the compute head
            nc.gpsimd.collective_compute(
                "AllGather", mybir.AluOpType.bypass, replica_groups=GROUPS,
                ins=[ag_warm_in[:]], outs=[ag_warm_out[:]])

            # chunk-0 q/k double-buffer through the ps_s slots and v(0)
            # through the (still idle) ps_pv slots -- two independent psum
            # chains run concurrently before attention(0) starts.
            spool = lambda: ps_s_pool.tile(
                [P, 2, 512], F32, tag="s", name="qkv0mm")[:, 0, :]
            vpool = lambda: ps_pv_pool.tile([P, 512], F32, tag="pv",
                                            name="qkv0v")
            for mt in range(NMT):
                for f in qk_emit(0, qT_tc[0], wq_sb, bq_sb, mt, pool=spool):
                    f()
            for mt in range(NMT):
                for f in qk_emit(0, kT_tc[0], wk_sb, bk_sb, mt, pool=spool):
                    f()
            for f in v_emit(0, 0, vpool=vpool):
                f()

            # per-chunk filler queues (half-group granularity), paced so
            # producers finish a few key-tiles before their consumers;
            # leftovers carry into the next chunk's queue.  proj(0)/proj(1)
            # ride as pinned fillers in c3/c4 (their AG completes mid-c3)
            # to keep the PE warm there; proj(2..4) form the tail backlog
            # that hides the last AllGather.
            f_c0 = [(V_COLS, f, False) for tt in (1, 2, 3)
                    for f in v_emit(0, tt)] + q_halves(1)
            _, rest = attention_chunk(0, f_c0)
            f_c1 = rest + k_halves(1) + v_halves(1) + q_halves(2)
            _, rest = attention_chunk(1, f_c1)
            f_c2 = rest + k_halves(2) + v_halves(2) + q_halves(3)
            _, rest = attention_chunk(2, f_c2)
            f_c3 = rest + k_halves(3) + v_halves(3) \
                + [(4096, g, True) for g in proj_groups(0)]
            _, rest = attention_chunk(3, f_c3)
            f_c4 = rest + [(4096, g, True) for g in proj_groups(1)]
            last_pv, _ = attention_chunk(4, f_c4, flush=True)
            # tail: proj(2..4) deferred behind the last AllGather trigger;
            # they run from the (now free) ps_s psum banks so consecutive
            # groups never serialize on a single-bank WAR.
            def tailpool():
                ps = ps_s_pool.tile([P, 2, 512], F32, tag="s",
                                    name="projps")
                return ps[:, 0, :]
            for ci in (2, 3, 4):
                for g in proj_groups(ci, pool=tailpool):
                    g(last_pv)

    nc.compile()
    return nc


_NC_CACHE = None


def _get_nc():
    global _NC_CACHE
    if _NC_CACHE is None:
        _NC_CACHE = build_bass()
    return _NC_CACHE


def _make_in_maps(x, Wqkv, bqkv, Wout, bout):
    bf16 = ml_dtypes.bfloat16
    in_maps = []
    for c in range(NCORES):
        b, g = c // 4, c % 4
        cs = DL * g  # column/dim slice start for this core's heads
        im = {
            "xt": np.ascontiguousarray(x[b].T).astype(bf16),
            "wq": np.ascontiguousarray(Wqkv[:, cs:cs + DL]).astype(bf16),
            "wk": np.ascontiguousarray(Wqkv[:, D + cs:D + cs + DL]).astype(bf16),
            "wv": np.ascontiguousarray(Wqkv[:, 2 * D + cs:2 * D + cs + DL]).astype(bf16),
            "wout": np.ascontiguousarray(Wout[:, cs:cs + DL]).astype(bf16),
            "bq": np.ascontiguousarray(
                bqkv[cs:cs + DL].reshape(NMT, P).T).astype(np.float32),
            "bk": np.ascontiguousarray(
                bqkv[D + cs:D + cs + DL].reshape(NMT, P).T).astype(np.float32),
            "bv": np.ascontiguousarray(np.broadcast_to(
                bqkv[2 * D + cs:2 * D + cs + DL].reshape(1, DL),
                (P, DL))).astype(np.float32),
            "bo": np.ascontiguousarray(
                bout[cs:cs + DL].reshape(NMT, P).T).astype(np.float32),
            "tri": np.triu(np.ones((P, P))).astype(bf16),
            "ones": np.ones((P, 32), dtype=bf16),
        }
        in_maps.append(im)
    return in_maps


def _run(inputs, trace=False, tmpdir=None):
    nc = _get_nc()
    in_maps = _make_in_maps(**inputs)
    res = bass_utils.run_bass_kernel_spmd(
        nc, in_maps, core_ids=list(range(NCORES)), trace=trace, tmpdir=tmpdir)
    out = np.empty((B, T, D), dtype=np.float32)
    for c in range(NCORES):
        b, g = c // 4, c % 4
        out[b, :, DL * g:DL * g + DL] = res.results[c]["outT"].T
    return out, res


def kernel(x, Wqkv, bqkv, Wout, bout):
    out, _ = _run(dict(x=np.asarray(x, dtype=np.float32),
                       Wqkv=np.asarray(Wqkv, dtype=np.float32),
                       bqkv=np.asarray(bqkv, dtype=np.float32),
                       Wout=np.asarray(Wout, dtype=np.float32),
                       bout=np.asarray(bout, dtype=np.float32)))
    return out
